# revision 2
# baseline (speedup 1.0000x reference)
"""Trainium2 Bass kernel for nn_AutoRegressiveInferenceNet (v3.1).

  logit = (2x-1) @ W0.T + b0                  [B, D]
  AR scan over D:  buf_i = (sigmoid(logit_i + W1[i] @ buf) > u_i)
  out = logit + (2 buf - 1) @ W1.T + b1
  returns (out, buf)

Sharding: data-parallel over batch across 8 NeuronCores (2048 rows/core),
W0/W1 replicated.  b0/b1 are zeros by construction: ignored.

v3.1 design:
  - negG (logistic(u) - logit) staged in DRAM, streamed per 16-col chunk.
  - 2 row groups of 8 row-tiles; group0's threshold prep is split into
    column halves so the scan starts after ~half the prep; group1's prep
    interleaves into the first scan slots (group1 scans DELTA chunks
    behind).  A zt-gate keeps the two DVE chains offset ~half a chunk.
  - per scan step: 2 fused DVE ops; chunk S materialized at chunk end.
  - cross-chunk coupling corrected per chunk by accumulating PE matmuls,
    applied on Pool from PSUM.
  - tail (group1-only chunks): group0's final matmuls run in fp32 straight
    from bufT, keeping PE warm; assembly on Pool.  End phase: only group1's
    finals in bf16.
"""
import sys
import numpy as np

sys.path.insert(0, "/opt/trn_rl_repo")

N_CORES = 8
B, IN, D = 16384, 1024, 1024
R = B // N_CORES          # 2048 rows per core
RT = R // 128             # 16 row tiles
G = 2
GRT = RT // G             # 8 row tiles per group
GR = R // G               # 1024 rows per group
CH = 16                   # scan chunk width
NCH = D // CH             # 64 chunks
BLK = 128
NBLK = D // BLK
CPB = BLK // CH           # 8 chunks per block
DELTA = 13                # group-1 scan lag (chunks)
GATE_J = 7                # group-0 step that opens group-1's chunk gate

_cached = None


def _build():
    import concourse.bass as bass
    import concourse.mybir as mybir
    import concourse.tile as tile
    from concourse import bacc
    from concourse.masks import make_identity

    dt = mybir.dt
    f32 = dt.float32
    bf16 = dt.bfloat16
    Alu = mybir.AluOpType
    Act = mybir.ActivationFunctionType

    nc = bacc.Bacc("TRN2", target_bir_lowering=False, debug=False,
                   num_devices=N_CORES)
    bf16 = dt.bfloat16

    x_ap = nc.dram_tensor("x", [R, IN], f32, kind="ExternalInput").ap()
    u_ap = nc.dram_tensor("u", [R, D], f32, kind="ExternalInput").ap()
    w0_ap = nc.dram_tensor("W0", [D, IN], f32, kind="ExternalInput").ap()
    w1_ap = nc.dram_tensor("W1", [D, D], f32, kind="ExternalInput").ap()
    out_ap = nc.dram_tensor("out", [R, D], f32, kind="ExternalOutput").ap()
    buf_ap = nc.dram_tensor("buf", [R, D], bf16,
                             kind="ExternalOutput").ap()
    lg_ap = nc.dram_tensor("lgscratch", [R, D], f32).ap()
    ng_aps = [[nc.dram_tensor(f"ngscratch{g}_{h}", [GR, 512], f32).ap()
               for h in range(2)] for g in range(G)]

    x_r = x_ap.rearrange("(t p) c -> p t c", p=128)
    u_r = u_ap.rearrange("(t p) c -> p t c", p=128)
    w0_r = w0_ap.rearrange("(t p) c -> p t c", p=128)
    w1_r = w1_ap.rearrange("(t p) c -> p t c", p=128)
    out_r = out_ap.rearrange("(t p) c -> p t c", p=128)
    buf_r = buf_ap.rearrange("(t p) c -> p t c", p=128)
    lg_r = lg_ap.rearrange("(t p) c -> p t c", p=128)
    ng_rs = [[a.rearrange("(t p) c -> p t c", p=128) for a in row]
             for row in ng_aps]

    with tile.TileContext(nc) as tc:
        with tc.tile_pool(name="pers", bufs=1) as pers:
            w0T = pers.tile([128, NBLK, D], f32)         # 32KB/p
            w1T = pers.tile([128, NBLK, D], f32)         # 32KB/p
            bufT = pers.tile([128, NBLK, R], f32)        # 64KB/p
            bufTb = pers.tile([128, NBLK, GR], bf16)     # 16KB/p (grp1 rows)
            w1s = pers.tile([128, D], f32)               # 4KB/p
            ident = pers.tile([128, 128], f32)
            make_identity(nc, ident[:])
            identb = pers.tile([128, 128], bf16)
            make_identity(nc, identb[:])

            io_pools = (tc.tile_pool(name="xio", bufs=2, side="right"),
                        tc.tile_pool(name="uio", bufs=1, side="right"),
                        tc.tile_pool(name="lps", bufs=1, space="PSUM",
                                     side="right"),
                        tc.tile_pool(name="tps", bufs=1, space="PSUM",
                                     side="right"))
            xio, uio, lps, tps = [p.__enter__() for p in io_pools]
            def emit_rt_prep(rt, g, nhs):
                """logit + threshold for column halves in nhs."""
                xp = xio.tile([128, IN], f32, tag="xp")
                nc.sync.dma_start(xp[:], x_r[:, rt, :])
                nc.scalar.activation(xp[:], xp[:], Act.Copy,
                                     bias=-1.0, scale=2.0)
                xT_ps = tps.tile([128, NBLK, 128], f32, tag="xTp")
                for kt in range(NBLK):
                    nc.tensor.transpose(
                        xT_ps[:, kt, :],
                        xp[:, kt * 128:(kt + 1) * 128], ident[:])
                xT = xio.tile([128, NBLK, 128], f32, tag="xT")
                nc.vector.tensor_copy(xT[:], xT_ps[:])
                for nh in nhs:
                    emit_half(rt, g, nh, xT)

            def emit_half(rt, g, nh, xT):
                cs = slice(nh * 512, (nh + 1) * 512)
                lp = lps.tile([128, 512], f32, tag="lp")
                for kt in range(NBLK):
                    nc.tensor.matmul(
                        lp[:], xT[:, kt, :], w0T[:, kt, cs],
                        start=(kt == 0), stop=(kt == NBLK - 1))
                ut = uio.tile([128, 512], f32, tag="ut")
                nc.sync.dma_start(ut[:], u_r[:, rt, cs])
                lu = uio.tile([128, 512], f32, tag="lu")
                nc.scalar.activation(lu[:], ut[:], Act.Ln)
                nc.scalar.activation(ut[:], ut[:], Act.Ln,
                                     bias=1.0, scale=-1.0)
                nc.gpsimd.tensor_tensor(lu[:], lu[:], ut[:], Alu.subtract)
                ngr = uio.tile([128, 512], f32, tag="ngr")
                nc.vector.scalar_tensor_tensor(
                    ngr[:], lp[:], -1.0, lu[:], Alu.mult, Alu.add)
                nc.sync.dma_start(
                    ng_rs[g][nh][:, rt - g * GRT, :], ngr[:])
                nc.scalar.copy(ut[:], lp[:])
                nc.sync.dma_start(lg_r[:, rt, cs], ut[:])


            def emit_w_prep(dst, src_r, pool, psum, tagp, kts):
                for ct in range(NBLK):
                    for kt in kts:
                        wt = pool.tile([128, 128], f32, tag=f"{tagp}w")
                        nc.sync.dma_start(
                            wt[:], src_r[:, kt, ct * 128:(ct + 1) * 128])
                        tp1 = psum.tile([128, 128], f32, tag="wt")
                        nc.tensor.transpose(tp1[:], wt[:], ident[:])
                        nc.scalar.copy(
                            dst[:, ct, kt * 128:(kt + 1) * 128], tp1[:])

            wpp_cm = tc.tile_pool(name="wprep", bufs=4, side="right")
            wpps_cm = tc.tile_pool(name="wpps", bufs=1, space="PSUM",
                                   side="right")
            wpp = wpp_cm.__enter__()
            wpps = wpps_cm.__enter__()
            # head: W0 output-cols 0:512, then group0 first-half prep,
            # then the first two W1 source-kt groups (needed by chunk 16).
            # Everything else is spread into the early scan slots below.
            emit_w_prep(w0T, w0_r, wpp, wpps, "w0", range(4))
            for rt in range(GRT):
                emit_rt_prep(rt, 0, [0])
            emit_w_prep(w1T, w1_r, wpp, wpps, "w1", [0, 1])

            def emit_spread(slot):
                # group-1 first-half prep on slots 0..7
                if slot < GRT:
                    emit_rt_prep(GRT + slot, 1, [0])
                # W1 source-kt groups 2..7 (kt=b needed by chunk 8b)
                if slot % 2 == 1 and 1 <= slot <= 11:
                    emit_w_prep(w1T, w1_r, wpp, wpps, "w1", [2 + slot // 2])
                # W0 second half at slots 12..15
                if 12 <= slot <= 15:
                    emit_w_prep(w0T, w0_r, wpp, wpps, "w0", [slot - 8])
                # second-half preps (need w0T cols 512:1024, ready ~slot 16):
                # group-0 on even slots 16..30, group-1 on odd slots 17..31
                if 16 <= slot <= 30 and slot % 2 == 0:
                    emit_rt_prep((slot - 16) // 2, 0, [1])
                if 17 <= slot <= 31 and slot % 2 == 1:
                    emit_rt_prep(GRT + (slot - 17) // 2, 1, [1])
                if slot == 32:
                    wpps_cm.__exit__(None, None, None)
                    wpp_cm.__exit__(None, None, None)

            # ---------------- scan ----------------
            with tc.tile_pool(name="scn", bufs=1) as scn, \
                 tc.tile_pool(name="ngp", bufs=3) as ngp, \
                 tc.tile_pool(name="wrp", bufs=2) as wrp, \
                 tc.tile_pool(name="pfp", bufs=1, space="PSUM") as pfp, \
                 tc.tile_pool(name="tbp", bufs=1, space="PSUM") as tbp:
                S_blk = [scn.tile([128, GRT, BLK], bf16, tag=f"S{g}",
                                  name=f"S_blk{g}")
                         for g in range(G)]
                tmp = [scn.tile([128, GRT, CH - 1], f32, tag=f"tmp{g}",
                                name=f"tmp{g}")
                       for g in range(G)]
                zt = scn.tile([128, 1], f32, tag="zt", name="zt")
                tb = tbp.tile([128, GR], bf16, tag="tb", name="tb")

                def emit_chunk(g, k):
                    r0 = g * GRT
                    b, m = divmod(k, CPB)
                    c0 = k * CH
                    wr = wrp.tile([128, CH, CH], f32, tag=f"wr{g}",
                                  name="wr")
                    nc.sync.dma_start(
                        wr[:],
                        w1_ap[c0:c0 + CH,
                              c0:c0 + CH].partition_broadcast(128))
                    ng = ngp.tile([128, GRT, CH], f32, tag=f"ng{g}",
                                  name="ng")
                    nh0, hc0 = divmod(c0, 512)
                    nc.sync.dma_start(
                        ng[:], ng_rs[g][nh0][:, :, hc0:hc0 + CH])
                    if k > 0:
                        pb, pm = divmod(k - 1, CPB)
                        W = CH * (pm + 1)
                        pf = pfp.tile([128, 2, GRT, CH], f32, tag=f"pf{g}",
                                      name="pf")

                        def kt_chains(dve_apply):
                            # contiguous accumulation chain per rl (a
                            # start=False matmul only accumulates onto the
                            # immediately-preceding chain's region)
                            for rl in range(GRT):
                                rs = (r0 + rl) * 128
                                for kt in range(b):
                                    nc.tensor.matmul(
                                        pf[:, 0, rl, :],
                                        bufT[:, kt, rs:rs + 128],
                                        w1T[:, kt, c0:c0 + CH],
                                        start=(kt == 0),
                                        stop=(kt == b - 1))
                            if dve_apply:
                                # on the critical path: one DVE op
                                nc.vector.scalar_tensor_tensor(
                                    ng[:], pf[:, 0], -1.0, ng[:],
                                    Alu.mult, Alu.add)
                                return
                            pfs0 = scn.tile([128, GRT, CH], f32,
                                            tag=f"pfs0{g}", name="pfs0",
                                            bufs=2)
                            nc.scalar.activation(pfs0[:], pf[:, 0],
                                                 Act.Copy, scale=-1.0)
                            nc.gpsimd.tensor_tensor(ng[:], ng[:], pfs0[:],
                                                    Alu.add)

                        if m > 0 and b > 0:
                            # old-block part: no dependency on this chunk's
                            # transposes/copy - runs early, off the chain
                            kt_chains(dve_apply=False)
                        for rl in range(GRT):
                            nc.tensor.transpose(
                                tb[0:W, rl * 128:(rl + 1) * 128],
                                S_blk[g][:, rl, 0:W], identb[:])
                        nc.scalar.copy(
                            bufT[0:W, pb, r0 * 128:r0 * 128 + GR],
                            tb[0:W, :])
                        if m > 0:
                            for rl in range(GRT):
                                rs = (r0 + rl) * 128
                                nc.tensor.matmul(
                                    pf[:, 1, rl, :],
                                    bufT[0:CH * m, b, rs:rs + 128],
                                    w1T[0:CH * m, b, c0:c0 + CH],
                                    start=True, stop=True)
                            # gating apply goes straight through DVE
                            # (PSUM src) - shortest path to the next chunk
                            nc.vector.scalar_tensor_tensor(
                                ng[:], pf[:, 1], -1.0, ng[:],
                                Alu.mult, Alu.add)
                        else:
                            kt_chains(dve_apply=True)

                    for j in range(CH - 1):
                        C = CH - 1 - j
                        zero = zt[:, 0:1] if (g == 1 and j == 0) else 0.0
                        nc.vector.scalar_tensor_tensor(
                            tmp[g][:, :, 0:C],
                            ng[:, :, j:j + 1].broadcast_to((128, GRT, C)),
                            zero,
                            wr[:, j + 1:CH, j:j + 1].rearrange(
                                "p a b -> p b a").broadcast_to(
                                (128, GRT, C)),
                            Alu.is_lt, Alu.mult)
                        nc.vector.tensor_tensor(
                            ng[:, :, j + 1:j + 1 + C],
                            ng[:, :, j + 1:j + 1 + C],
                            tmp[g][:, :, 0:C], Alu.subtract)
                        if g == 0 and j == GATE_J:
                            nc.vector.tensor_scalar(
                                zt[:], ng[:, 0, j + 1:j + 2],
                                0.0, None, Alu.mult)
                    nc.vector.tensor_scalar(
                        S_blk[g][:, :, m * CH:(m + 1) * CH],
                        ng[:], 0.0, None, Alu.is_lt)
                    nc.sync.dma_start(
                        buf_r[:, r0:r0 + GRT, c0:c0 + CH],
                        S_blk[g][:, :, m * CH:(m + 1) * CH])

                def emit_drain(g, k_last):
                    r0 = g * GRT
                    pb, pm = divmod(k_last, CPB)
                    W = CH * (pm + 1)
                    for rl in range(GRT):
                        nc.tensor.transpose(
                            tb[0:W, rl * 128:(rl + 1) * 128],
                            S_blk[g][:, rl, 0:W], identb[:])
                    nc.scalar.copy(
                        bufT[0:W, pb, r0 * 128:r0 * 128 + GR],
                        tb[0:W, :])

                g0fin_state = {}

                def emit_g0_final_setup():
                    # close prep pools, open tail pools (PSUM freed)
                    for p in reversed(io_pools):
                        p.__exit__(None, None, None)
                    g0fin_state["sb_cm"] = tc.tile_pool(name="g0f", bufs=2)
                    g0fin_state["ps_cm"] = tc.tile_pool(name="g0fp", bufs=1,
                                                        space="PSUM")
                    g0fin_state["sb"] = g0fin_state["sb_cm"].__enter__()
                    g0fin_state["ps"] = g0fin_state["ps_cm"].__enter__()
                    sb, ps = g0fin_state["sb"], g0fin_state["ps"]
                    ones = sb.tile([128, 128], f32, tag="ones")
                    nc.gpsimd.memset(ones[:], 1.0)
                    ws = ps.tile([128, D], f32, tag="ws")
                    for ct in range(NBLK):
                        for nh in range(2):
                            nc.tensor.matmul(
                                ws[:, nh * 512:(nh + 1) * 512],
                                ones[:],
                                w1T[:, ct, nh * 512:(nh + 1) * 512],
                                start=(ct == 0), stop=(ct == NBLK - 1))
                    nc.scalar.copy(w1s[:], ws[:])

                def emit_g0_final_rt(rt):
                    sb, ps = g0fin_state["sb"], g0fin_state["ps"]
                    lgt = sb.tile([128, D], f32, tag="lgt")
                    nc.sync.dma_start(lgt[:], lg_r[:, rt, :])
                    fp = ps.tile([128, D], f32, tag="fp")
                    for ct in range(NBLK):
                        for nh in range(2):
                            nc.tensor.matmul(
                                fp[:, nh * 512:(nh + 1) * 512],
                                bufT[:, ct, rt * 128:(rt + 1) * 128],
                                w1T[:, ct, nh * 512:(nh + 1) * 512],
                                start=(ct == 0), stop=(ct == NBLK - 1))
                    # ot = 2*fp + (lgt - w1s): ACT moves fp off PSUM,
                    # Pool does the arithmetic (GPSIMD cannot touch PSUM)
                    nc.gpsimd.tensor_tensor(lgt[:], lgt[:], w1s[:],
                                            Alu.subtract)
                    fpc = sb.tile([128, D], f32, tag="fpc")
                    nc.scalar.activation(fpc[:], fp[:], Act.Copy, scale=2.0)
                    ot = sb.tile([128, D], f32, tag="ot")
                    nc.gpsimd.tensor_tensor(ot[:], fpc[:], lgt[:], Alu.add)
                    nc.sync.dma_start(out_r[:, rt, :], ot[:])

                for slot in range(NCH + DELTA):
                    if slot < NCH:
                        emit_chunk(0, slot)
                    elif slot == NCH:
                        emit_drain(0, NCH - 1)
                        emit_g0_final_setup()
                    emit_spread(slot)
                    # group-0 finals fill the tail
                    if slot > NCH and (slot - NCH) % 2 == 0:
                        rt = (slot - NCH) // 2 - 1
                        if rt < GRT:
                            emit_g0_final_rt(rt)
                    k1 = slot - DELTA
                    if 0 <= k1 < NCH:
                        emit_chunk(1, k1)
                    # bf16 convert of group1's completed blocks
                    if slot > DELTA and (slot - DELTA) % CPB == 0:
                        done = (slot - DELTA) // CPB - 1
                        if 0 <= done < NBLK - 1:
                            nc.gpsimd.tensor_copy(bufTb[:, done, :],
                                                  bufT[:, done, GR:R])
                emit_drain(1, NCH - 1)
                nc.gpsimd.tensor_copy(bufTb[:, NBLK - 1, :],
                                      bufT[:, NBLK - 1, GR:R])
                # remaining group-0 finals (if tail slots ran out)
                for rt in range((DELTA - 1) // 2, GRT):
                    emit_g0_final_rt(rt)
                g0fin_state["ps_cm"].__exit__(None, None, None)
                g0fin_state["sb_cm"].__exit__(None, None, None)

            # ---------------- end: group1 finals in bf16 ----------------
            w1Tb = w1T[:].bitcast(bf16)
            nc.gpsimd.tensor_copy(w1Tb[:, :, 0:D], w1T[:])
            with tc.tile_pool(name="fin", bufs=2) as fin, \
                 tc.tile_pool(name="fps", bufs=2, space="PSUM") as fps:
                for rl in range(GRT):
                    rt = GRT + rl
                    lgt = fin.tile([128, D], f32, tag="lgt")
                    nc.sync.dma_start(lgt[:], lg_r[:, rt, :])
                    fp = fps.tile([128, D], f32, tag="fp")
                    for ct in range(NBLK):
                        for nh in range(2):
                            nc.tensor.matmul(
                                fp[:, nh * 512:(nh + 1) * 512],
                                bufTb[:, ct, rl * 128:(rl + 1) * 128],
                                w1Tb[:, ct, nh * 512:(nh + 1) * 512],
                                start=(ct == 0), stop=(ct == NBLK - 1))
                    nc.gpsimd.tensor_tensor(lgt[:], lgt[:], w1s[:],
                                            Alu.subtract)
                    ot = fin.tile([128, D], f32, tag="ot")
                    nc.vector.scalar_tensor_tensor(
                        ot[:], fp[:], 2.0, lgt[:], Alu.mult, Alu.add)
                    nc.sync.dma_start(out_r[:, rt, :], ot[:])

    nc.compile()
    return nc


def _get_nc():
    global _cached
    if _cached is None:
        _cached = _build()
    return _cached


def kernel(x, W0, b0, W1, b1, u):
    from concourse.bass_utils import run_bass_kernel_spmd

    nc = _get_nc()
    x = np.ascontiguousarray(np.asarray(x, np.float32))
    u = np.ascontiguousarray(np.asarray(u, np.float32))
    W0 = np.ascontiguousarray(np.asarray(W0, np.float32))
    W1 = np.ascontiguousarray(np.asarray(W1, np.float32))
    in_maps = []
    for c in range(N_CORES):
        sl = slice(c * R, (c + 1) * R)
        in_maps.append({"x": x[sl], "u": u[sl], "W0": W0, "W1": W1})
    res = run_bass_kernel_spmd(nc, in_maps, core_ids=list(range(N_CORES)))
    out = np.concatenate([res.results[c]["out"] for c in range(N_CORES)], 0)
    buf = np.concatenate([np.asarray(res.results[c]["buf"], np.float32)
                          for c in range(N_CORES)], 0)
    return out, buf


# revision 3
# speedup vs baseline: 1.0014x; 1.0014x over previous
"""Trainium2 Bass kernel for nn_AutoRegressiveInferenceNet (v3.1).

  logit = (2x-1) @ W0.T + b0                  [B, D]
  AR scan over D:  buf_i = (sigmoid(logit_i + W1[i] @ buf) > u_i)
  out = logit + (2 buf - 1) @ W1.T + b1
  returns (out, buf)

Sharding: data-parallel over batch across 8 NeuronCores (2048 rows/core),
W0/W1 replicated.  b0/b1 are zeros by construction: ignored.

v3.1 design:
  - negG (logistic(u) - logit) staged in DRAM, streamed per 16-col chunk.
  - 2 row groups of 8 row-tiles; group0's threshold prep is split into
    column halves so the scan starts after ~half the prep; group1's prep
    interleaves into the first scan slots (group1 scans DELTA chunks
    behind).  A zt-gate keeps the two DVE chains offset ~half a chunk.
  - per scan step: 2 fused DVE ops; chunk S materialized at chunk end.
  - cross-chunk coupling corrected per chunk by accumulating PE matmuls,
    applied on Pool from PSUM.
  - tail (group1-only chunks): group0's final matmuls run in fp32 straight
    from bufT, keeping PE warm; assembly on Pool.  End phase: only group1's
    finals in bf16.
"""
import sys
import numpy as np

sys.path.insert(0, "/opt/trn_rl_repo")

N_CORES = 8
B, IN, D = 16384, 1024, 1024
R = B // N_CORES          # 2048 rows per core
RT = R // 128             # 16 row tiles
G = 2
GRT = RT // G             # 8 row tiles per group
GR = R // G               # 1024 rows per group
CH = 16                   # scan chunk width
NCH = D // CH             # 64 chunks
BLK = 128
NBLK = D // BLK
CPB = BLK // CH           # 8 chunks per block
DELTA = 11                # group-1 scan lag (chunks)
GATE_J = 7                # group-0 step that opens group-1's chunk gate

_cached = None


def _build():
    import concourse.bass as bass
    import concourse.mybir as mybir
    import concourse.tile as tile
    from concourse import bacc
    from concourse.masks import make_identity

    dt = mybir.dt
    f32 = dt.float32
    bf16 = dt.bfloat16
    Alu = mybir.AluOpType
    Act = mybir.ActivationFunctionType

    nc = bacc.Bacc("TRN2", target_bir_lowering=False, debug=False,
                   num_devices=N_CORES)
    bf16 = dt.bfloat16

    x_ap = nc.dram_tensor("x", [R, IN], f32, kind="ExternalInput").ap()
    u_ap = nc.dram_tensor("u", [R, D], f32, kind="ExternalInput").ap()
    w0_ap = nc.dram_tensor("W0", [D, IN], f32, kind="ExternalInput").ap()
    w1_ap = nc.dram_tensor("W1", [D, D], f32, kind="ExternalInput").ap()
    out_ap = nc.dram_tensor("out", [R, D], f32, kind="ExternalOutput").ap()
    buf_ap = nc.dram_tensor("buf", [R, D], bf16,
                             kind="ExternalOutput").ap()
    lg_ap = nc.dram_tensor("lgscratch", [R, D], f32).ap()
    ng_aps = [[nc.dram_tensor(f"ngscratch{g}_{h}", [GR, 512], f32).ap()
               for h in range(2)] for g in range(G)]

    x_r = x_ap.rearrange("(t p) c -> p t c", p=128)
    u_r = u_ap.rearrange("(t p) c -> p t c", p=128)
    w0_r = w0_ap.rearrange("(t p) c -> p t c", p=128)
    w1_r = w1_ap.rearrange("(t p) c -> p t c", p=128)
    out_r = out_ap.rearrange("(t p) c -> p t c", p=128)
    buf_r = buf_ap.rearrange("(t p) c -> p t c", p=128)
    lg_r = lg_ap.rearrange("(t p) c -> p t c", p=128)
    ng_rs = [[a.rearrange("(t p) c -> p t c", p=128) for a in row]
             for row in ng_aps]

    with tile.TileContext(nc) as tc:
        with tc.tile_pool(name="pers", bufs=1) as pers:
            w0T = pers.tile([128, NBLK, D], f32)         # 32KB/p
            w1T = pers.tile([128, NBLK, D], f32)         # 32KB/p
            bufT = pers.tile([128, NBLK, R], f32)        # 64KB/p
            bufTb = pers.tile([128, NBLK, GR], bf16)     # 16KB/p (grp1 rows)
            w1s = pers.tile([128, D], f32)               # 4KB/p
            ident = pers.tile([128, 128], f32)
            make_identity(nc, ident[:])
            identb = pers.tile([128, 128], bf16)
            make_identity(nc, identb[:])

            io_pools = (tc.tile_pool(name="xio", bufs=2, side="right"),
                        tc.tile_pool(name="uio", bufs=1, side="right"),
                        tc.tile_pool(name="lps", bufs=1, space="PSUM",
                                     side="right"),
                        tc.tile_pool(name="tps", bufs=1, space="PSUM",
                                     side="right"))
            xio, uio, lps, tps = [p.__enter__() for p in io_pools]
            def emit_rt_prep(rt, g, nhs):
                """logit + threshold for column halves in nhs."""
                xp = xio.tile([128, IN], f32, tag="xp")
                nc.sync.dma_start(xp[:], x_r[:, rt, :])
                nc.scalar.activation(xp[:], xp[:], Act.Copy,
                                     bias=-1.0, scale=2.0)
                xT_ps = tps.tile([128, NBLK, 128], f32, tag="xTp")
                for kt in range(NBLK):
                    nc.tensor.transpose(
                        xT_ps[:, kt, :],
                        xp[:, kt * 128:(kt + 1) * 128], ident[:])
                xT = xio.tile([128, NBLK, 128], f32, tag="xT")
                nc.vector.tensor_copy(xT[:], xT_ps[:])
                for nh in nhs:
                    emit_half(rt, g, nh, xT)

            def emit_half(rt, g, nh, xT):
                cs = slice(nh * 512, (nh + 1) * 512)
                lp = lps.tile([128, 512], f32, tag="lp")
                for kt in range(NBLK):
                    nc.tensor.matmul(
                        lp[:], xT[:, kt, :], w0T[:, kt, cs],
                        start=(kt == 0), stop=(kt == NBLK - 1))
                ut = uio.tile([128, 512], f32, tag="ut")
                nc.sync.dma_start(ut[:], u_r[:, rt, cs])
                lu = uio.tile([128, 512], f32, tag="lu")
                nc.scalar.activation(lu[:], ut[:], Act.Ln)
                nc.scalar.activation(ut[:], ut[:], Act.Ln,
                                     bias=1.0, scale=-1.0)
                nc.gpsimd.tensor_tensor(lu[:], lu[:], ut[:], Alu.subtract)
                ngr = uio.tile([128, 512], f32, tag="ngr")
                nc.vector.scalar_tensor_tensor(
                    ngr[:], lp[:], -1.0, lu[:], Alu.mult, Alu.add)
                nc.sync.dma_start(
                    ng_rs[g][nh][:, rt - g * GRT, :], ngr[:])
                nc.scalar.copy(ut[:], lp[:])
                nc.sync.dma_start(lg_r[:, rt, cs], ut[:])


            def emit_w_prep(dst, src_r, pool, psum, tagp, kts):
                for ct in range(NBLK):
                    for kt in kts:
                        wt = pool.tile([128, 128], f32, tag=f"{tagp}w")
                        nc.sync.dma_start(
                            wt[:], src_r[:, kt, ct * 128:(ct + 1) * 128])
                        tp1 = psum.tile([128, 128], f32, tag="wt")
                        nc.tensor.transpose(tp1[:], wt[:], ident[:])
                        nc.scalar.copy(
                            dst[:, ct, kt * 128:(kt + 1) * 128], tp1[:])

            wpp_cm = tc.tile_pool(name="wprep", bufs=4, side="right")
            wpps_cm = tc.tile_pool(name="wpps", bufs=1, space="PSUM",
                                   side="right")
            wpp = wpp_cm.__enter__()
            wpps = wpps_cm.__enter__()
            # head: W0 output-cols 0:512, then group0 first-half prep,
            # then the first two W1 source-kt groups (needed by chunk 16).
            # Everything else is spread into the early scan slots below.
            emit_w_prep(w0T, w0_r, wpp, wpps, "w0", range(4))
            for rt in range(GRT):
                emit_rt_prep(rt, 0, [0])
            emit_w_prep(w1T, w1_r, wpp, wpps, "w1", [0, 1])

            def emit_spread(slot):
                # group-1 first-half prep on slots 0..7
                if slot < GRT:
                    emit_rt_prep(GRT + slot, 1, [0])
                # W1 source-kt groups 2..7 (kt=b needed by chunk 8b)
                if slot % 2 == 1 and 1 <= slot <= 11:
                    emit_w_prep(w1T, w1_r, wpp, wpps, "w1", [2 + slot // 2])
                # W0 second half at slots 12..15
                if 12 <= slot <= 15:
                    emit_w_prep(w0T, w0_r, wpp, wpps, "w0", [slot - 8])
                # second-half preps (need w0T cols 512:1024, ready ~slot 16):
                # group-0 on even slots 16..30, group-1 on odd slots 17..31
                if 16 <= slot <= 30 and slot % 2 == 0:
                    emit_rt_prep((slot - 16) // 2, 0, [1])
                if 17 <= slot <= 31 and slot % 2 == 1:
                    emit_rt_prep(GRT + (slot - 17) // 2, 1, [1])
                if slot == 32:
                    wpps_cm.__exit__(None, None, None)
                    wpp_cm.__exit__(None, None, None)

            # ---------------- scan ----------------
            with tc.tile_pool(name="scn", bufs=1) as scn, \
                 tc.tile_pool(name="ngp", bufs=3) as ngp, \
                 tc.tile_pool(name="wrp", bufs=2) as wrp, \
                 tc.tile_pool(name="pfp", bufs=1, space="PSUM") as pfp, \
                 tc.tile_pool(name="tbp", bufs=1, space="PSUM") as tbp:
                S_blk = [scn.tile([128, GRT, BLK], bf16, tag=f"S{g}",
                                  name=f"S_blk{g}")
                         for g in range(G)]
                tmp = [scn.tile([128, GRT, CH - 1], f32, tag=f"tmp{g}",
                                name=f"tmp{g}")
                       for g in range(G)]
                zt = scn.tile([128, 1], f32, tag="zt", name="zt")
                tb = tbp.tile([128, GR], bf16, tag="tb", name="tb")

                def emit_chunk(g, k):
                    r0 = g * GRT
                    b, m = divmod(k, CPB)
                    c0 = k * CH
                    wr = wrp.tile([128, CH, CH], f32, tag=f"wr{g}",
                                  name="wr")
                    nc.sync.dma_start(
                        wr[:],
                        w1_ap[c0:c0 + CH,
                              c0:c0 + CH].partition_broadcast(128))
                    ng = ngp.tile([128, GRT, CH], f32, tag=f"ng{g}",
                                  name="ng")
                    nh0, hc0 = divmod(c0, 512)
                    nc.sync.dma_start(
                        ng[:], ng_rs[g][nh0][:, :, hc0:hc0 + CH])
                    if k > 0:
                        pb, pm = divmod(k - 1, CPB)
                        W = CH * (pm + 1)
                        pf = pfp.tile([128, 2, GRT, CH], f32, tag=f"pf{g}",
                                      name="pf")

                        def kt_chains(dve_apply):
                            # contiguous accumulation chain per rl (a
                            # start=False matmul only accumulates onto the
                            # immediately-preceding chain's region)
                            for rl in range(GRT):
                                rs = (r0 + rl) * 128
                                for kt in range(b):
                                    nc.tensor.matmul(
                                        pf[:, 0, rl, :],
                                        bufT[:, kt, rs:rs + 128],
                                        w1T[:, kt, c0:c0 + CH],
                                        start=(kt == 0),
                                        stop=(kt == b - 1))
                            if dve_apply:
                                # on the critical path: one DVE op
                                nc.vector.scalar_tensor_tensor(
                                    ng[:], pf[:, 0], -1.0, ng[:],
                                    Alu.mult, Alu.add)
                                return
                            pfs0 = scn.tile([128, GRT, CH], f32,
                                            tag=f"pfs0{g}", name="pfs0",
                                            bufs=2)
                            nc.scalar.activation(pfs0[:], pf[:, 0],
                                                 Act.Copy, scale=-1.0)
                            nc.gpsimd.tensor_tensor(ng[:], ng[:], pfs0[:],
                                                    Alu.add)

                        if m > 0 and b > 0:
                            # old-block part: no dependency on this chunk's
                            # transposes/copy - runs early, off the chain
                            kt_chains(dve_apply=False)
                        for rl in range(GRT):
                            nc.tensor.transpose(
                                tb[0:W, rl * 128:(rl + 1) * 128],
                                S_blk[g][:, rl, 0:W], identb[:])
                        nc.scalar.copy(
                            bufT[0:W, pb, r0 * 128:r0 * 128 + GR],
                            tb[0:W, :])
                        if m > 0:
                            for rl in range(GRT):
                                rs = (r0 + rl) * 128
                                nc.tensor.matmul(
                                    pf[:, 1, rl, :],
                                    bufT[0:CH * m, b, rs:rs + 128],
                                    w1T[0:CH * m, b, c0:c0 + CH],
                                    start=True, stop=True)
                            # gating apply goes straight through DVE
                            # (PSUM src) - shortest path to the next chunk
                            nc.vector.scalar_tensor_tensor(
                                ng[:], pf[:, 1], -1.0, ng[:],
                                Alu.mult, Alu.add)
                        else:
                            kt_chains(dve_apply=True)

                    for j in range(CH - 1):
                        C = CH - 1 - j
                        zero = zt[:, 0:1] if (g == 1 and j == 0) else 0.0
                        nc.vector.scalar_tensor_tensor(
                            tmp[g][:, :, 0:C],
                            ng[:, :, j:j + 1].broadcast_to((128, GRT, C)),
                            zero,
                            wr[:, j + 1:CH, j:j + 1].rearrange(
                                "p a b -> p b a").broadcast_to(
                                (128, GRT, C)),
                            Alu.is_lt, Alu.mult)
                        nc.vector.tensor_tensor(
                            ng[:, :, j + 1:j + 1 + C],
                            ng[:, :, j + 1:j + 1 + C],
                            tmp[g][:, :, 0:C], Alu.subtract)
                        if g == 0 and j == GATE_J:
                            nc.vector.tensor_scalar(
                                zt[:], ng[:, 0, j + 1:j + 2],
                                0.0, None, Alu.mult)
                    nc.vector.tensor_scalar(
                        S_blk[g][:, :, m * CH:(m + 1) * CH],
                        ng[:], 0.0, None, Alu.is_lt)
                    nc.sync.dma_start(
                        buf_r[:, r0:r0 + GRT, c0:c0 + CH],
                        S_blk[g][:, :, m * CH:(m + 1) * CH])

                def emit_drain(g, k_last):
                    r0 = g * GRT
                    pb, pm = divmod(k_last, CPB)
                    W = CH * (pm + 1)
                    for rl in range(GRT):
                        nc.tensor.transpose(
                            tb[0:W, rl * 128:(rl + 1) * 128],
                            S_blk[g][:, rl, 0:W], identb[:])
                    nc.scalar.copy(
                        bufT[0:W, pb, r0 * 128:r0 * 128 + GR],
                        tb[0:W, :])

                g0fin_state = {}

                def emit_g0_final_setup():
                    # close prep pools, open tail pools (PSUM freed)
                    for p in reversed(io_pools):
                        p.__exit__(None, None, None)
                    g0fin_state["sb_cm"] = tc.tile_pool(name="g0f", bufs=2)
                    g0fin_state["ps_cm"] = tc.tile_pool(name="g0fp", bufs=1,
                                                        space="PSUM")
                    g0fin_state["sb"] = g0fin_state["sb_cm"].__enter__()
                    g0fin_state["ps"] = g0fin_state["ps_cm"].__enter__()
                    sb, ps = g0fin_state["sb"], g0fin_state["ps"]
                    ones = sb.tile([128, 128], f32, tag="ones")
                    nc.gpsimd.memset(ones[:], 1.0)
                    ws = ps.tile([128, D], f32, tag="ws")
                    for ct in range(NBLK):
                        for nh in range(2):
                            nc.tensor.matmul(
                                ws[:, nh * 512:(nh + 1) * 512],
                                ones[:],
                                w1T[:, ct, nh * 512:(nh + 1) * 512],
                                start=(ct == 0), stop=(ct == NBLK - 1))
                    nc.scalar.copy(w1s[:], ws[:])

                def emit_g0_final_rt(rt):
                    sb, ps = g0fin_state["sb"], g0fin_state["ps"]
                    lgt = sb.tile([128, D], f32, tag="lgt")
                    nc.sync.dma_start(lgt[:], lg_r[:, rt, :])
                    fp = ps.tile([128, D], f32, tag="fp")
                    for ct in range(NBLK):
                        for nh in range(2):
                            nc.tensor.matmul(
                                fp[:, nh * 512:(nh + 1) * 512],
                                bufT[:, ct, rt * 128:(rt + 1) * 128],
                                w1T[:, ct, nh * 512:(nh + 1) * 512],
                                start=(ct == 0), stop=(ct == NBLK - 1))
                    # ot = 2*fp + (lgt - w1s): ACT moves fp off PSUM,
                    # Pool does the arithmetic (GPSIMD cannot touch PSUM)
                    nc.gpsimd.tensor_tensor(lgt[:], lgt[:], w1s[:],
                                            Alu.subtract)
                    fpc = sb.tile([128, D], f32, tag="fpc")
                    nc.scalar.activation(fpc[:], fp[:], Act.Copy, scale=2.0)
                    ot = sb.tile([128, D], f32, tag="ot")
                    nc.gpsimd.tensor_tensor(ot[:], fpc[:], lgt[:], Alu.add)
                    nc.sync.dma_start(out_r[:, rt, :], ot[:])

                for slot in range(NCH + DELTA):
                    if slot < NCH:
                        emit_chunk(0, slot)
                    elif slot == NCH:
                        emit_drain(0, NCH - 1)
                        emit_g0_final_setup()
                    emit_spread(slot)
                    # group-0 finals fill the tail
                    if slot > NCH and (slot - NCH) % 2 == 0:
                        rt = (slot - NCH) // 2 - 1
                        if rt < GRT:
                            emit_g0_final_rt(rt)
                    k1 = slot - DELTA
                    if 0 <= k1 < NCH:
                        emit_chunk(1, k1)
                    # bf16 convert of group1's completed blocks
                    if slot > DELTA and (slot - DELTA) % CPB == 0:
                        done = (slot - DELTA) // CPB - 1
                        if 0 <= done < NBLK - 1:
                            nc.gpsimd.tensor_copy(bufTb[:, done, :],
                                                  bufT[:, done, GR:R])
                emit_drain(1, NCH - 1)
                nc.gpsimd.tensor_copy(bufTb[:, NBLK - 1, :],
                                      bufT[:, NBLK - 1, GR:R])
                # remaining group-0 finals (if tail slots ran out)
                for rt in range(max(0, (DELTA - 1) // 2), GRT):
                    emit_g0_final_rt(rt)
                g0fin_state["ps_cm"].__exit__(None, None, None)
                g0fin_state["sb_cm"].__exit__(None, None, None)

            # ---------------- end: group1 finals in bf16 ----------------
            w1Tb = w1T[:].bitcast(bf16)
            for ct in range(NBLK):
                nc.gpsimd.tensor_copy(w1Tb[:, ct, 0:D],
                                      w1T[:, ct, :])
            with tc.tile_pool(name="fin", bufs=3) as fin, \
                 tc.tile_pool(name="fps", bufs=2, space="PSUM") as fps:
                # fold "- w1s" into each matmul chain: one extra matmul
                # with a constant -0.5/128 stationary against replicated
                # w1s adds -0.5*colsum; ot = 2*fp + lgt then matches
                # logit - colsum + 2*(buf@W1T)
                mhalf = fin.tile([128, 128], bf16, tag="mhalf")
                nc.gpsimd.memset(mhalf[:], -0.00390625)
                w1sb = fin.tile([128, D], bf16, tag="w1sb")
                nc.scalar.copy(w1sb[:], w1s[:])
                for rl in range(GRT):
                    rt = GRT + rl
                    lgt = fin.tile([128, D], f32, tag="lgt")
                    nc.sync.dma_start(lgt[:], lg_r[:, rt, :])
                    fp = fps.tile([128, D], f32, tag="fp")
                    for nh in range(2):
                        for ct in range(NBLK):
                            nc.tensor.matmul(
                                fp[:, nh * 512:(nh + 1) * 512],
                                bufTb[:, ct, rl * 128:(rl + 1) * 128],
                                w1Tb[:, ct, nh * 512:(nh + 1) * 512],
                                start=(ct == 0), stop=False)
                        nc.tensor.matmul(
                            fp[:, nh * 512:(nh + 1) * 512],
                            mhalf[:],
                            w1sb[:, nh * 512:(nh + 1) * 512],
                            start=False, stop=True)
                    ot = fin.tile([128, D], f32, tag="ot")
                    nc.vector.scalar_tensor_tensor(
                        ot[:], fp[:], 2.0, lgt[:], Alu.mult, Alu.add)
                    nc.sync.dma_start(out_r[:, rt, :], ot[:])

    nc.compile()
    return nc


def _get_nc():
    global _cached
    if _cached is None:
        _cached = _build()
    return _cached


def kernel(x, W0, b0, W1, b1, u):
    from concourse.bass_utils import run_bass_kernel_spmd

    nc = _get_nc()
    x = np.ascontiguousarray(np.asarray(x, np.float32))
    u = np.ascontiguousarray(np.asarray(u, np.float32))
    W0 = np.ascontiguousarray(np.asarray(W0, np.float32))
    W1 = np.ascontiguousarray(np.asarray(W1, np.float32))
    in_maps = []
    for c in range(N_CORES):
        sl = slice(c * R, (c + 1) * R)
        in_maps.append({"x": x[sl], "u": u[sl], "W0": W0, "W1": W1})
    res = run_bass_kernel_spmd(nc, in_maps, core_ids=list(range(N_CORES)))
    out = np.concatenate([res.results[c]["out"] for c in range(N_CORES)], 0)
    buf = np.concatenate([np.asarray(res.results[c]["buf"], np.float32)
                          for c in range(N_CORES)], 0)
    return out, buf
